# revision 14
# baseline (speedup 1.0000x reference)
"""DenseNGCN layer on 8 trn2 NeuronCores.

  x = features @ weight                    [50000, 512] @ [512, 64]
  x = A @ x   (twice, A sparse COO E=800k: segment_sum(val * x[col], row))
  out = x + bias

Strategy (dst-node sharding; gather-descriptor generation is the bottleneck):
  - Nodes sharded across 8 cores (6250 rows each, padded to 6272 = 49 blocks
    of 128). Each core owns the projection + SpMM rows of its shard.
  - The SpMM is a per-dst-block dma_gather of deduped source PAIRS (256 B
    elements from the bf16 [N,64] replica) followed by dense matmuls with a
    host-precomputed scatter matrix S (two parity halves per chunk).
  - The serial cost is GPSIMD Q7 descriptor generation (~7 ns/descriptor),
    so this version attacks descriptor count and Q7 idle time:
      * prepare_only + trigger_dma: descriptor generation is decoupled from
        data readiness, so the Q7 pipelines through collectives/matmul tails
        and never idles waiting for the AllGather.
      * exact per-block slot counts (cross-core max, 128-aligned) instead of
        a global worst-case KCH grid.
      * an "arena" of the ACH*128 highest-usage pairs per core is gathered
        ONCE per pass and applied to every dst block with extra matmuls,
        removing those pairs from every per-block gather (~25% fewer
        descriptors at ACH=27 for extra PE + S traffic, both of which hide
        under the Q7 descriptor wall).
  - Full x is re-replicated to every core's HBM between SpMM iterations via
    an AllGather collective (bf16, Shared-scratchpad output).

All edge metadata (slot lists, wrapped gather indices, dense S tensors) is
precomputed host-side into per-core tensors; the device program is identical
across cores (SPMD).
"""

import numpy as np
import ml_dtypes

N = 50000
E = 800000
IN_CH = 512
OUT_CH = 64
C = 8
P = 128
NSHARD = N // C                # 6250
BLKS = (NSHARD + P - 1) // P   # 49
NPAD = BLKS * P                # 6272
NTOT = NPAD * C                # 50176
NPAIR = NTOT // 2              # 25088

MB = 4      # dst blocks per merged gather call
ACH = 27    # arena chunks (128 pairs each); 0 disables the arena
SPLIT = 28  # blocks in half A; half-major row layout so the AllGather of
HA = SPLIT * P          # half A (3584 rows/core) can overlap half B's compute
HB = NPAD - HA          # 2688

_CACHE = {}


def _wrap16(idxs):
    """[n] -> [16, n//16] with idx j at [j % 16, j // 16]."""
    return np.ascontiguousarray(idxs.reshape(-1, 16).T)


def _prep(adj_indices, adj_values):
    row = adj_indices[0].astype(np.int64)
    col = adj_indices[1].astype(np.int64)
    val = adj_values.astype(np.float32)

    core = row // NSHARD
    loc = row % NSHARD
    blk = loc // P
    rl = loc % P
    # half-major global layout: [all cores' blocks 0..SPLIT-1 | all cores' rest]
    ccol = col // NSHARD
    lcol = col % NSHARD
    pcol = np.where(lcol < HA, ccol * HA + lcol, C * HA + ccol * HB + (lcol - HA))
    pair = pcol >> 1
    parity = (pcol & 1).astype(np.int64)

    # distinct (core, blk, pair) and per-(core, pair) usage
    key = (core * BLKS + blk) * NPAIR + pair
    uniq = np.unique(key)
    u_core = uniq // (BLKS * NPAIR)
    u_blk = (uniq // NPAIR) % BLKS
    u_pair = uniq % NPAIR

    K = ACH * P
    lut = np.full((C, NPAIR), -1, np.int32)  # (core, pair) -> arena slot
    if K:
        cp = u_core * NPAIR + u_pair
        cp_uniq, cp_cnt = np.unique(cp, return_counts=True)
        cpc = cp_uniq // NPAIR
        cpp = cp_uniq % NPAIR
        # per core: top-K pairs by usage (ties by pair id), then pair-sorted
        order = np.lexsort((cpp, -cp_cnt, cpc))
        arena_pairs = np.empty((C, K), np.int64)
        start = np.searchsorted(cpc[order], np.arange(C))
        for c in range(C):
            sel = order[start[c] : start[c] + K]
            ap = np.sort(cpp[sel])
            arena_pairs[c] = ap
            lut[c, ap] = np.arange(K, dtype=np.int32)

    # split distinct slots / edges into arena vs per-block
    u_arena = lut[u_core, u_pair] >= 0
    e_arena = lut[core, pair] >= 0

    # per-(core, blk) slot lists (non-arena), sorted by pair
    kc, kb, kp = u_core[~u_arena], u_blk[~u_arena], u_pair[~u_arena]
    cnt = np.bincount(kc * BLKS + kb, minlength=C * BLKS).reshape(C, BLKS)
    m_b = ((cnt.max(axis=0) + P - 1) // P * P).astype(np.int64)  # common

    # slot tables: slots[c][b] padded with pair 0 to m_b[b]
    ord2 = np.lexsort((kp, kb, kc))
    kc, kb, kp = kc[ord2], kb[ord2], kp[ord2]
    starts = np.searchsorted(kc * BLKS + kb, np.arange(C * BLKS))
    ends = np.append(starts[1:], len(kc))
    slot_tab = np.zeros((C, BLKS, int(m_b.max())), np.int64)
    for c in range(C):
        for b in range(BLKS):
            s, e = starts[c * BLKS + b], ends[c * BLKS + b]
            slot_tab[c, b, : e - s] = kp[s:e]

    # call structure (identical across cores)
    calls = [list(range(m, min(m + MB, BLKS))) for m in range(0, BLKS, MB)]
    call_n = [int(m_b[bs].sum()) for bs in calls]
    tile_base = np.zeros(BLKS, np.int64)  # first S tile of each block
    t = 0
    for b in range(BLKS):
        tile_base[b] = t
        t += (m_b[b] // P) * 2
    reg_tiles = int(t)

    # gather index grids: [arena | call 0 | call 1 | ...], wrapped + replicated
    gi_parts = []
    for c in range(C):
        parts = []
        if K:
            parts.append(_wrap16(arena_pairs[c]))
        for bs in calls:
            idxs = np.concatenate([slot_tab[c, b, : m_b[b]] for b in bs])
            parts.append(_wrap16(idxs))
        gi_parts.append(np.concatenate(parts, axis=1))
    gi = np.stack(gi_parts).astype(np.int16)          # [C, 16, GI_COLS]
    gall = np.tile(gi, (1, 8, 1))                      # [C, 128, GI_COLS]

    # S for per-block slots: order block -> chunk -> parity, [128, 128] tiles
    # edge lane = position of its pair in slot_tab[c, b]; col = tile*P + rl
    s_host = np.zeros((C, P, reg_tiles * P), np.float32)
    ec, eb, ep = core[~e_arena], blk[~e_arena], pair[~e_arena]
    epar, erl, ev = parity[~e_arena], rl[~e_arena], val[~e_arena]
    pos = np.empty(len(ec), np.int64)
    for c in range(C):
        for b in range(BLKS):
            m = (ec == c) & (eb == b)
            if m.any():
                pos[m] = np.searchsorted(slot_tab[c, b, : cnt[c, b]], ep[m])
    tile_idx = tile_base[eb] + (pos // P) * 2 + epar
    np.add.at(s_host, (ec, pos % P, tile_idx * P + erl), ev)
    s_host = s_host.astype(ml_dtypes.bfloat16)

    # arena S: order block -> chunk -> parity, [128, 128] tiles
    if K:
        sa_host = np.zeros((C, P, BLKS * ACH * 2 * P), np.float32)
        aslot = lut[core, pair][e_arena].astype(np.int64)
        ac, ab = core[e_arena], blk[e_arena]
        apar, arl, av = parity[e_arena], rl[e_arena], val[e_arena]
        atile = (ab * ACH + aslot // P) * 2 + apar
        np.add.at(sa_host, (ac, aslot % P, atile * P + arl), av)
        sa_host = sa_host.astype(ml_dtypes.bfloat16)
    else:
        sa_host = np.zeros((C, P, 0), ml_dtypes.bfloat16)

    meta = {
        "m_b": [int(x) for x in m_b],
        "calls": calls,
        "call_n": call_n,
        "gi_cols": gall.shape[2],
        "reg_tiles": reg_tiles,
        "tile_base": [int(x) for x in tile_base],
    }
    return meta, gall, s_host, sa_host


def _build(meta):
    import concourse.bacc as bacc
    import concourse.mybir as mybir
    from concourse import tile

    f32 = mybir.dt.float32
    bf16 = mybir.dt.bfloat16
    i16 = mybir.dt.int16

    m_b = meta["m_b"]
    calls = meta["calls"]
    call_n = meta["call_n"]
    reg_tiles = meta["reg_tiles"]
    tile_base = meta["tile_base"]
    GMAX = max(call_n) // P
    SMAX = max(
        sum((m_b[b] // P) * 2 for b in bs) for bs in calls
    )

    nc = bacc.Bacc(
        None,
        target_bir_lowering=False,
        num_devices=C,
        dynamic_dma_scratch_size=1 << 16,
    )

    featT_d = nc.dram_tensor("featT", [IN_CH, NPAD], bf16, kind="ExternalInput")
    w_d = nc.dram_tensor("w", [IN_CH, OUT_CH], bf16, kind="ExternalInput")
    bias_d = nc.dram_tensor("bias", [P, OUT_CH], f32, kind="ExternalInput")
    gi_d = nc.dram_tensor("gi", [P, meta["gi_cols"]], i16, kind="ExternalInput")
    s_d = nc.dram_tensor("s", [P, reg_tiles * P], bf16, kind="ExternalInput")
    if ACH:
        sa_d = nc.dram_tensor(
            "sa", [P, BLKS * ACH * 2 * P], bf16, kind="ExternalInput"
        )
    out_d = nc.dram_tensor("out", [NPAD, OUT_CH], f32, kind="ExternalOutput")

    xshA_d = nc.dram_tensor("x_shard_a", [HA, OUT_CH], bf16)
    xshB_d = nc.dram_tensor("x_shard_b", [HB, OUT_CH], bf16)
    xA_d = nc.dram_tensor("xA", [NTOT, OUT_CH], bf16)
    xB_d = nc.dram_tensor("xB", [NTOT, OUT_CH], bf16)

    with tile.TileContext(nc) as tc:
        with (
            tc.tile_pool(name="const", bufs=1) as cpool,
            tc.tile_pool(name="g", bufs=3) as gpool,
            tc.tile_pool(name="ga", bufs=2) as apool,
            tc.tile_pool(name="s", bufs=2) as spool,
            tc.tile_pool(name="sa", bufs=2) as sapool,
            tc.tile_pool(name="o", bufs=3) as opool,
            tc.tile_pool(name="psum", bufs=8, space="PSUM") as pp,
        ):
            w_sb = cpool.tile([P, IN_CH // P, OUT_CH], bf16)
            bias_sb = cpool.tile([P, OUT_CH], f32)
            gi_sb = cpool.tile([P, meta["gi_cols"]], i16)
            xA_sb = cpool.tile([P, SPLIT, OUT_CH], bf16)
            xB_sb = cpool.tile([P, BLKS - SPLIT, OUT_CH], bf16)

            def copy_block(b, ps):
                if b < SPLIT:
                    nc.vector.tensor_copy(xA_sb[:, b, :], ps[:])
                else:
                    nc.vector.tensor_copy(xB_sb[:, b - SPLIT, :], ps[:])

            def store_half_a():
                nc.sync.dma_start(
                    xshA_d[:].rearrange("(b p) c -> p b c", p=P), xA_sb[:]
                )

            def store_half_b():
                nc.sync.dma_start(
                    xshB_d[:].rearrange("(b p) c -> p b c", p=P), xB_sb[:]
                )

            def allgather_a(dst):
                nc.gpsimd.collective_compute(
                    "AllGather",
                    mybir.AluOpType.bypass,
                    replica_groups=[list(range(C))],
                    ins=[xshA_d[:]],
                    outs=[dst[: C * HA, :]],
                )

            def allgather_b(dst):
                nc.gpsimd.collective_compute(
                    "AllGather",
                    mybir.AluOpType.bypass,
                    replica_groups=[list(range(C))],
                    ins=[xshB_d[:]],
                    outs=[dst[C * HA :, :]],
                )

            nc.sync.dma_start(w_sb[:], w_d[:].rearrange("(k p) c -> p k c", p=P))
            nc.sync.dma_start(bias_sb[:], bias_d[:])
            nc.scalar.dma_start(gi_sb[:], gi_d[:])

            # --- projection: x0 = features @ W for this core's rows ---
            GRP = 7  # blocks per feature-tile group (49 = 7*7)
            with tc.tile_pool(name="feat", bufs=2) as fpool:
                for g in range(BLKS // GRP):
                    feat_sb = fpool.tile([P, IN_CH // P, GRP * P], bf16, tag="f")
                    eng = nc.sync if g % 2 == 0 else nc.scalar
                    eng.dma_start(
                        feat_sb[:],
                        featT_d[:, g * GRP * P : (g + 1) * GRP * P].rearrange(
                            "(k p) n -> p k n", p=P
                        ),
                    )
                    for bb in range(GRP):
                        b = g * GRP + bb
                        ps = pp.tile([P, OUT_CH], f32, tag="ps")
                        for k in range(IN_CH // P):
                            nc.tensor.matmul(
                                ps[:],
                                feat_sb[:, k, bb * P : (bb + 1) * P],
                                w_sb[:, k, :],
                                start=(k == 0),
                                stop=(k == IN_CH // P - 1),
                            )
                        copy_block(b, ps)
                    if g * GRP + GRP == SPLIT:
                        store_half_a()
                        allgather_a(xA_d)
                store_half_b()
                allgather_b(xA_d)

            def gather(src_pairs, gi_off, n, out_ap):
                nc.gpsimd.dma_gather(
                    out_ap,
                    src_pairs,
                    gi_sb[:, gi_off : gi_off + n // 16],
                    n,
                    n,
                    2 * OUT_CH,
                    single_packet=False,
                )

            def spmm(src, last):
                # pair view: row i = x[2i] ++ x[2i+1], 256 B
                src_pairs = src[:].rearrange("(a b) c -> a (b c)", b=2)
                gi_off = 0
                if ACH:
                    Ga = apool.tile([P, ACH, 2 * OUT_CH], bf16, tag="Ga")
                    gather(src_pairs, 0, ACH * P, Ga[:])
                    gi_off = ACH * P // 16
                for ci, bs in enumerate(calls):
                    nch = call_n[ci] // P
                    G = gpool.tile([P, GMAX, 2 * OUT_CH], bf16, tag="G")
                    gather(src_pairs, gi_off, call_n[ci], G[:, :nch, :])
                    gi_off += call_n[ci] // 16
                    ntile = sum((m_b[b] // P) * 2 for b in bs)
                    S = spool.tile([P, SMAX * P], bf16, tag="S")
                    eng = nc.sync if ci % 2 == 0 else nc.scalar
                    eng.dma_start(
                        S[:, : ntile * P],
                        s_d[:, tile_base[bs[0]] * P : (tile_base[bs[0]] + ntile) * P],
                    )
                    goff = 0
                    for b in bs:
                        ps = pp.tile([P, OUT_CH], f32, tag="ps")
                        first = True
                        if ACH:
                            Sa = sapool.tile([P, ACH * 2 * P], bf16, tag="Sa")
                            eng2 = nc.scalar if ci % 2 == 0 else nc.sync
                            eng2.dma_start(
                                Sa[:],
                                sa_d[:, b * ACH * 2 * P : (b + 1) * ACH * 2 * P],
                            )
                            for ch in range(ACH):
                                for par in range(2):
                                    nc.tensor.matmul(
                                        ps[:],
                                        Sa[:, (ch * 2 + par) * P : (ch * 2 + par + 1) * P],
                                        Ga[:, ch, par * OUT_CH : (par + 1) * OUT_CH],
                                        start=first,
                                        stop=False,
                                    )
                                    first = False
                        nbch = m_b[b] // P
                        st = tile_base[b] - tile_base[bs[0]]
                        for ch in range(nbch):
                            for par in range(2):
                                nc.tensor.matmul(
                                    ps[:],
                                    S[:, (st + ch * 2 + par) * P : (st + ch * 2 + par + 1) * P],
                                    G[:, goff + ch, par * OUT_CH : (par + 1) * OUT_CH],
                                    start=first,
                                    stop=(ch == nbch - 1 and par == 1),
                                )
                                first = False
                        goff += nbch
                        if last:
                            o = opool.tile([P, OUT_CH], f32, tag="o")
                            nc.vector.tensor_tensor(
                                o[:], ps[:], bias_sb[:], mybir.AluOpType.add
                            )
                            nc.scalar.dma_start(
                                out_d[b * P : (b + 1) * P, :], o[:]
                            )
                        else:
                            copy_block(b, ps)
                    if not last and bs[-1] == SPLIT - 1:
                        store_half_a()
                        allgather_a(xB_d)
                if not last:
                    store_half_b()
                    allgather_b(xB_d)

            spmm(xA_d, last=False)
            spmm(xB_d, last=True)

    nc.compile()
    return nc


LAST_RESULT = None


def kernel(adj_indices, adj_values, features, weight, bias):
    global LAST_RESULT
    from concourse.bass_utils import run_bass_kernel_spmd

    meta, gall, s_host, sa_host = _prep(
        np.asarray(adj_indices), np.asarray(adj_values)
    )

    ck = (tuple(meta["m_b"]), ACH)
    if ck not in _CACHE:
        _CACHE[ck] = _build(meta)
    nc = _CACHE[ck]

    features = np.asarray(features, np.float32)
    weight = np.ascontiguousarray(
        np.asarray(weight, np.float32).astype(ml_dtypes.bfloat16)
    )
    bias128 = np.tile(np.asarray(bias, np.float32).reshape(1, OUT_CH), (P, 1))

    in_maps = []
    for c in range(C):
        featT = np.zeros((IN_CH, NPAD), ml_dtypes.bfloat16)
        featT[:, :NSHARD] = (
            features[c * NSHARD : (c + 1) * NSHARD].T.astype(ml_dtypes.bfloat16)
        )
        im = {
            "featT": featT,
            "w": weight,
            "bias": bias128,
            "gi": np.ascontiguousarray(gall[c]),
            "s": s_host[c],
        }
        if ACH:
            im["sa"] = sa_host[c]
        in_maps.append(im)

    res = run_bass_kernel_spmd(nc, in_maps, core_ids=list(range(C)))
    LAST_RESULT = res

    out = np.concatenate(
        [res.results[c]["out"][:NSHARD] for c in range(C)], axis=0
    )
    return out
